# revision 1
# baseline (speedup 1.0000x reference)
"""Trainium2 Bass kernel for the GRU caption model.

Computes: h0 = feat @ W_hp.T + b_hp; 200-step GRU with constant hidden-proj
gate pre-activations; logits = outs @ W_out.T + b_out -> [B, V, T].

Strategy: every core runs the (tiny, latency-bound) GRU redundantly; the
vocab dimension of W_out is sharded 8 ways; each core emits its own
[B, 3840, T] logits slice which the host concatenates.

All on-chip compute uses a transposed [feature-on-partitions, batch-free]
layout so the recurrent state feeds the next step's matmul directly.
"""

import numpy as np
import ml_dtypes

import concourse.bass as bass
import concourse.mybir as mybir
import concourse.tile as tile
from concourse import bacc
from concourse.bass_utils import run_bass_kernel_spmd

F32 = mybir.dt.float32
F32R = mybir.dt.float32r
BF16 = mybir.dt.bfloat16
AF = mybir.ActivationFunctionType
ALU = mybir.AluOpType

VOCAB = 30522
HID = 512
FEAT = 2048
STEPS = 200
BATCH = 32
SOS = 101
NCORES = 8
P = 128
KO = HID // P          # 4 h-chunks
GM = 3 * HID // P      # 12 gate row-groups (r: 0-3, z: 4-7, n: 8-11)
KF = FEAT // P         # 16 feat chunks
VPAD = 3840            # per-core padded vocab rows = 30 * 128
MT = VPAD // P         # 30 vocab tiles per core
TBLOCKS = [(0, 64), (64, 128), (128, 200)]  # proj t-blocks

LAST_RESULTS = None  # test harness introspection
EMIT_GRU = True    # variant switch (sim experiments)
EMIT_PROJ = True   # variant switch (sim experiments)
PROJ_MODE = 2      # 0 = matmuls only, 1 = +copies, 2 = +DMA (sim experiments)


def _r(ap):
    """Reinterpret an fp32 AP as float32r for full-rate PE streaming."""
    return ap.bitcast(F32R)


def build():
    nc = bacc.Bacc("TRN2", target_bir_lowering=False, debug=False)

    featT = nc.dram_tensor("featT", [FEAT, BATCH], F32, kind="ExternalInput")
    WhpT = nc.dram_tensor("WhpT", [FEAT, HID], F32, kind="ExternalInput")
    WihT = nc.dram_tensor("WihT", [HID, 3 * HID], BF16, kind="ExternalInput")
    WhhT = nc.dram_tensor("WhhT", [HID, 3 * HID], F32, kind="ExternalInput")
    b_ih = nc.dram_tensor("b_ih", [3 * HID], F32, kind="ExternalInput")
    b_hh = nc.dram_tensor("b_hh", [3 * HID], F32, kind="ExternalInput")
    b_hp = nc.dram_tensor("b_hp", [HID], F32, kind="ExternalInput")
    x0T = nc.dram_tensor("x0T", [HID, BATCH], BF16, kind="ExternalInput")
    WoutT = nc.dram_tensor("WoutT", [HID, VPAD], F32R, kind="ExternalInput")
    b_out = nc.dram_tensor("b_out", [VPAD], F32, kind="ExternalInput")
    OUT = nc.dram_tensor("OUT", [BATCH, VPAD, STEPS], F32, kind="ExternalOutput")

    with tile.TileContext(nc) as tc:
        with (
            tc.tile_pool(name="const", bufs=1) as const,
            tc.tile_pool(name="stream", bufs=3) as stream,
            tc.tile_pool(name="step", bufs=4) as sp,
            tc.tile_pool(name="hb", bufs=4) as hb,
            tc.tile_pool(name="outp", bufs=6) as outp,
            tc.tile_pool(name="psg", bufs=3, space="PSUM") as psg,
            tc.tile_pool(name="psp", bufs=4, space="PSUM") as psp,
        ):
            # ---- constants into SBUF ----
            wih = const.tile([P, KO, GM, P], BF16, tag="wih")
            nc.sync.dma_start(
                wih[:], WihT.rearrange("(k p) (m c) -> p k m c", p=P, c=P)
            )
            featT_sb = const.tile([P, KF, BATCH], F32, tag="featsb")
            nc.sync.dma_start(featT_sb[:], featT.rearrange("(k p) b -> p k b", p=P))
            bih_sb = const.tile([P, GM], F32, tag="bih")
            nc.sync.dma_start(bih_sb[:], b_ih.rearrange("(m p) -> p m", p=P))
            bhh_sb = const.tile([P, GM], F32, tag="bhh")
            nc.sync.dma_start(bhh_sb[:], b_hh.rearrange("(m p) -> p m", p=P))
            bhp_sb = const.tile([P, KO], F32, tag="bhp")
            nc.sync.dma_start(bhp_sb[:], b_hp.rearrange("(m p) -> p m", p=P))
            bout_sb = const.tile([P, MT], F32, tag="bout")
            nc.sync.dma_start(bout_sb[:], b_out.rearrange("(m p) -> p m", p=P))

            WhpT_r = WhpT.rearrange("(k p) h -> p k h", p=P)
            WhhT_r = WhhT.rearrange("(k p) g -> p k g", p=P)
            WoutT_r = WoutT.rearrange("(k p) v -> p k v", p=P)

            # ---- h0 = feat @ W_hp.T + b_hp (fp32, exact) ----
            ps_h = psg.tile([P, GM, BATCH], F32, tag="gates")
            for ko in range(KO):
                for kf in range(KF):
                    wt = stream.tile([P, P], F32, tag="whp")
                    nc.sync.dma_start(wt[:], WhpT_r[:, kf, ko * P:(ko + 1) * P])
                    nc.tensor.matmul(
                        ps_h[:, ko, :], wt[:], featT_sb[:, kf, :],
                        start=(kf == 0), stop=(kf == KF - 1),
                    )
            h0T = const.tile([P, KO, BATCH], F32, tag="h0T")
            for ko in range(KO):
                nc.scalar.activation(
                    h0T[:, ko, :], ps_h[:, ko, :], AF.Identity,
                    bias=bhp_sb[:, ko, None], scale=1.0,
                )
            h0_half = const.tile([P, KO, BATCH], F32, tag="h0h")
            nc.scalar.mul(h0_half[:], h0T[:], 0.5)

            # ---- gh = h0 @ W_hh.T + b_hh (fp32, exact; step-invariant) ----
            ps_g = psg.tile([P, GM, BATCH], F32, tag="gates")
            for m in range(GM):
                for k in range(KO):
                    wt = stream.tile([P, P], F32, tag="whh")
                    nc.sync.dma_start(wt[:], WhhT_r[:, k, m * P:(m + 1) * P])
                    nc.tensor.matmul(
                        ps_g[:, m, :], wt[:], h0T[:, k, :],
                        start=(k == 0), stop=(k == KO - 1),
                    )
            ghT = const.tile([P, GM, BATCH], F32, tag="ghT")
            for m in range(GM):
                nc.scalar.activation(
                    ghT[:, m, :], ps_g[:, m, :], AF.Identity,
                    bias=bhh_sb[:, m, None], scale=1.0,
                )
            # C_rz = gh_rz + b_ih_rz ; hn2 = 0.5*gh_n ; E_n = hn2 + b_ih_n
            C_rz = const.tile([P, 8, BATCH], F32, tag="Crz")
            nc.vector.tensor_add(
                C_rz[:], ghT[:, 0:8, :],
                bih_sb[:, 0:8, None].to_broadcast((P, 8, BATCH)),
            )
            hn2 = const.tile([P, KO, BATCH], F32, tag="hn2")
            nc.scalar.mul(hn2[:], ghT[:, 8:12, :], 0.5)
            E_n = const.tile([P, KO, BATCH], F32, tag="En")
            nc.vector.tensor_add(
                E_n[:], hn2[:],
                bih_sb[:, 8:12, None].to_broadcast((P, KO, BATCH)),
            )

            # resT blocks: col = b*bsize + (t - t0), per h-chunk ko
            resT = []
            for j, (t0, t1) in enumerate(TBLOCKS):
                bs = t1 - t0
                rt = const.tile(
                    [P, KO, BATCH, bs], F32R, tag=f"resT{j}", name=f"resT{j}"
                )
                resT.append(rt)

            prev = hb.tile([P, KO, BATCH], BF16, tag="hb")
            nc.sync.dma_start(prev[:], x0T.rearrange("(k p) b -> p k b", p=P))

            def proj_block(j):
                t0, t1 = TBLOCKS[j]
                bs = t1 - t0
                gb = 4
                N = gb * bs
                for m in range(MT):
                    wt = stream.tile([P, KO, P], F32R, tag="wout")
                    nc.sync.dma_start(wt[:], WoutT_r[:, :, m * P:(m + 1) * P])
                    for g in range(BATCH // gb):
                        ps_full = psp.tile([P, 288], F32, tag="pp", name="pp")
                        ps = ps_full[:, :N]
                        for k in range(KO):
                            nc.tensor.matmul(
                                ps,
                                wt[:, k, :],
                                resT[j][:, k, gb * g:gb * g + gb, :],
                                start=(k == 0), stop=(k == KO - 1),
                            )
                        if PROJ_MODE == 0:
                            continue
                        ob_full = outp.tile([P, 288], F32, tag="ob", name="ob")
                        ob = ob_full[:, :N]
                        if (m + g) % 2 == 0:
                            nc.scalar.activation(
                                ob, ps, AF.Identity,
                                bias=bout_sb[:, m, None], scale=1.0,
                            )
                        else:
                            nc.vector.tensor_scalar_add(ob, ps, bout_sb[:, m, None])
                        if PROJ_MODE >= 2:
                            dst = OUT[
                                gb * g:gb * g + gb, m * P:(m + 1) * P, t0:t1
                            ].rearrange("b v t -> v b t")
                            nc.sync.dma_start(
                                dst, ob.rearrange("p (b t) -> p b t", b=gb)
                            )

            # ---- GRU steps ----
            if not EMIT_GRU:
                for j in range(len(TBLOCKS)):
                    nc.vector.memset(resT[j][:], 0.25)
                    proj_block(j)
            mm_order = [8, 9, 10, 11] + list(range(8))  # n-gates first
            for t in range(STEPS if EMIT_GRU else 0):
                ps = psg.tile([P, GM, BATCH], F32, tag="gates")
                for m in mm_order:
                    for k in range(KO):
                        nc.tensor.matmul(
                            ps[:, m, :], wih[:, k, m, :], prev[:, k, :],
                            start=(k == 0), stop=(k == KO - 1),
                        )
                s_rz = sp.tile([P, 8, BATCH], F32, tag="srz")
                nc.vector.tensor_add(s_rz[:], ps[:, 0:8, :], C_rz[:])
                t_rz = sp.tile([P, 8, BATCH], F32, tag="trz")
                nc.scalar.activation(t_rz[:], s_rz[:], AF.Tanh, scale=0.5)
                a = sp.tile([P, KO, BATCH], F32, tag="a")
                nc.vector.tensor_mul(a[:], t_rz[:, 0:4, :], hn2[:])
                sn1 = sp.tile([P, KO, BATCH], F32, tag="sn1")
                nc.vector.tensor_add(sn1[:], ps[:, 8:12, :], E_n[:])
                sn2 = sp.tile([P, KO, BATCH], F32, tag="sn2")
                nc.vector.tensor_add(sn2[:], sn1[:], a[:])
                n = sp.tile([P, KO, BATCH], F32, tag="n")
                nc.scalar.activation(n[:], sn2[:], AF.Tanh, scale=1.0)
                q = sp.tile([P, KO, BATCH], F32, tag="q")
                nc.vector.tensor_sub(q[:], h0T[:], n[:])
                w2 = sp.tile([P, KO, BATCH], F32, tag="w2")
                nc.vector.scalar_tensor_tensor(
                    w2[:], t_rz[:, 4:8, :], 0.5, q[:], ALU.mult, ALU.mult
                )
                p2 = sp.tile([P, KO, BATCH], F32, tag="p2")
                nc.vector.scalar_tensor_tensor(
                    p2[:], n[:], 0.5, h0_half[:], ALU.mult, ALU.add
                )
                nxt = hb.tile([P, KO, BATCH], BF16, tag="hb")
                nc.vector.tensor_add(nxt[:], w2[:], p2[:])
                j = next(i for i, (a, b) in enumerate(TBLOCKS) if a <= t < b)
                t0 = TBLOCKS[j][0]
                nc.gpsimd.tensor_add(resT[j][:, :, :, t - t0], w2[:], p2[:])
                prev = nxt
                if t == TBLOCKS[j][1] - 1 and EMIT_PROJ:
                    proj_block(j)

    nc.compile()
    return nc


def _shard_inputs(feat, W_hp, b_hp, W_ih, W_hh, b_ih, b_hh, embed, W_out, b_out):
    bf = ml_dtypes.bfloat16
    featT = np.ascontiguousarray(feat.T, dtype=np.float32)
    WhpT = np.ascontiguousarray(W_hp.T, dtype=np.float32)
    WihT = np.ascontiguousarray(W_ih.T).astype(bf)
    WhhT = np.ascontiguousarray(W_hh.T, dtype=np.float32)
    x0T = np.ascontiguousarray(
        np.repeat(np.asarray(embed)[SOS][:, None], BATCH, axis=1)
    ).astype(bf)
    Wo = np.zeros((NCORES * VPAD, HID), np.float32)
    Wo[:VOCAB] = W_out
    bo = np.zeros((NCORES * VPAD,), np.float32)
    bo[:VOCAB] = b_out
    common = dict(
        featT=featT, WhpT=WhpT, WihT=WihT, WhhT=WhhT,
        b_ih=np.asarray(b_ih, np.float32), b_hh=np.asarray(b_hh, np.float32),
        b_hp=np.asarray(b_hp, np.float32), x0T=x0T,
    )
    in_maps = []
    for c in range(NCORES):
        sl = slice(c * VPAD, (c + 1) * VPAD)
        m = dict(common)
        m["WoutT"] = np.ascontiguousarray(Wo[sl].T)
        m["b_out"] = bo[sl].copy()
        in_maps.append(m)
    return in_maps


def kernel(**inputs):
    global LAST_RESULTS
    args = {k: np.asarray(v) for k, v in inputs.items()}
    in_maps = _shard_inputs(
        args["feat"], args["W_hp"], args["b_hp"], args["W_ih"], args["W_hh"],
        args["b_ih"], args["b_hh"], args["embed"], args["W_out"], args["b_out"],
    )
    nc = build()
    res = run_bass_kernel_spmd(nc, in_maps, core_ids=list(range(NCORES)))
    LAST_RESULTS = res
    out = np.concatenate([r["OUT"] for r in res.results], axis=1)[:, :VOCAB, :]
    return np.ascontiguousarray(out, dtype=np.float32)



# revision 5
# speedup vs baseline: 1.6964x; 1.6964x over previous
"""Trainium2 Bass kernel for the GRU caption model.

Computes: h0 = feat @ W_hp.T + b_hp; 200-step GRU whose hidden-proj gate
pre-activations are step-invariant; logits = outs @ W_out.T + b_out -> [B, V, T].

Strategy (v2):
- Every core runs the (tiny, latency-bound) GRU redundantly; the vocab dim of
  W_out/b_out is sharded 8 ways; each core emits a [3840, T, B]-ish fp16 logits
  shard which the host transposes/concatenates.
- All recurrent state and 2-byte operands are fp16 (1 cyc/row matmuls, 2x/4x
  DVE modes, half DMA bytes) while h0/gh are computed exactly in fp32 and the
  step-invariant gate constants are PRE-LOADED into PSUM each step so the
  matmuls accumulate on top (start=False) - this removes the "+C" adds from
  the recurrent critical path.
- Per step the chain is: r-matmuls -> tanh(r) -> mul -> add -> tanh(n) ->
  2 fused scalar_tensor_tensor ops -> h' (written straight into the fp16
  time-major state buffer that both the next step's matmuls and the vocab
  projection read).
- The vocab projection (the bulk of PE work) is chopped into (16-step x 30
  vocab-tile) units and interleaved between GRU steps so PE never idles; its
  PSUM tiles are copied (+bias, fp16-cast) to SBUF on Act/DVE alternately and
  written to DRAM with a handful of large, fully-coalesced DMAs.
"""

import collections

import numpy as np
import ml_dtypes

import concourse.bass as bass
import concourse.mybir as mybir
import concourse.tile as tile
from concourse import bacc
from concourse.bass_utils import run_bass_kernel_spmd

F32 = mybir.dt.float32
F16 = mybir.dt.float16
AF = mybir.ActivationFunctionType
ALU = mybir.AluOpType

VOCAB = 30522
HID = 512
FEAT = 2048
STEPS = 200
B = 32
SOS = 101
NCORES = 8
P = 128
KO = HID // P           # 4 h-chunks
GM = 3 * HID // P       # 12 gate row-groups (r: 0-3, z: 4-7, n: 8-11)
KF = FEAT // P          # 16 feat chunks
VPAD = 3840             # per-core padded vocab rows = 30 * 128
MT = VPAD // P          # 30 vocab tiles per core

# projection column groups: 16-step slabs (last one 8)
GROUPS = [(t, min(t + 16, STEPS)) for t in range(0, STEPS, 16)]

LAST_RESULTS = None  # test harness introspection


def build():
    nc = bacc.Bacc("TRN2", target_bir_lowering=False, debug=False)

    FEATP = nc.dram_tensor("FEATP", [P, KF, B], F32, kind="ExternalInput")
    WHPP = nc.dram_tensor("WHPP", [P, KF, HID], F32, kind="ExternalInput")
    WHHP = nc.dram_tensor("WHHP", [P, KO, 3 * HID], F32, kind="ExternalInput")
    WIHP = nc.dram_tensor("WIHP", [P, KO, GM, P], F16, kind="ExternalInput")
    WOUTP = nc.dram_tensor("WOUTP", [P, KO, VPAD], F16, kind="ExternalInput")
    BIHP = nc.dram_tensor("BIHP", [P, GM], F32, kind="ExternalInput")
    BHHP = nc.dram_tensor("BHHP", [P, GM], F32, kind="ExternalInput")
    BHPP = nc.dram_tensor("BHPP", [P, KO], F32, kind="ExternalInput")
    BOUTP = nc.dram_tensor("BOUTP", [P, MT], F32, kind="ExternalInput")
    X0P = nc.dram_tensor("X0P", [P, KO, B], F16, kind="ExternalInput")
    OUT = nc.dram_tensor("OUT", [P, MT, STEPS, B], F16, kind="ExternalOutput")

    with tile.TileContext(nc) as tc:
        with (
            tc.tile_pool(name="const", bufs=1) as const,
            tc.tile_pool(name="stream", bufs=2) as stream,
            tc.tile_pool(name="step", bufs=3) as sp,
            tc.tile_pool(name="obp", bufs=2) as obp,
            tc.tile_pool(name="psg", bufs=3, space="PSUM") as psg,
            tc.tile_pool(name="psp", bufs=4, space="PSUM") as psp,
        ):
            # ---- resident constants ----
            wih = const.tile([P, KO, GM, P], F16, tag="wih")
            nc.sync.dma_start(wih[:], WIHP[:, :, :, :])
            wout = const.tile([P, KO, VPAD], F16, tag="wout")
            nc.sync.dma_start(wout[:], WOUTP[:, :, :])
            featT = const.tile([P, KF, B], F32, tag="featT")
            nc.sync.dma_start(featT[:], FEATP[:, :, :])
            bih_sb = const.tile([P, GM], F32, tag="bih")
            nc.sync.dma_start(bih_sb[:], BIHP[:, :])
            bhh_sb = const.tile([P, GM], F32, tag="bhh")
            nc.sync.dma_start(bhh_sb[:], BHHP[:, :])
            bhp_sb = const.tile([P, KO], F32, tag="bhp")
            nc.sync.dma_start(bhp_sb[:], BHPP[:, :])
            bout_sb = const.tile([P, MT], F32, tag="bout")
            nc.sync.dma_start(bout_sb[:], BOUTP[:, :])

            # recurrent state, time-major: resB[:, t+1] = h_t ; resB[:, 0] = x0
            resB = const.tile([P, STEPS + 1, KO, B], F16, tag="resB")
            nc.sync.dma_start(resB[:, 0, :, :], X0P[:, :, :])

            # ---- h0 = feat @ W_hp.T + b_hp (fp32, exact) ----
            # b_hp is PRE-LOADED into PSUM; all matmuls accumulate (start=False)
            ps_h = psg.tile([P, 16, B], F32, tag="gates")
            nc.vector.tensor_copy(
                ps_h[:, 0:KO, :], bhp_sb[:, :, None].to_broadcast((P, KO, B))
            )
            for kc in range(4):
                wchunk = stream.tile([P, 4, HID], F32, tag="wst")
                nc.sync.dma_start(wchunk[:], WHPP[:, 4 * kc:4 * kc + 4, :])
                for j in range(4):
                    kf = 4 * kc + j
                    for ko in range(KO):
                        nc.tensor.matmul(
                            ps_h[:, ko, :],
                            wchunk[:, j, ko * P:(ko + 1) * P],
                            featT[:, kf, :],
                            start=False, stop=(kf == KF - 1),
                            skip_group_check=True,
                        )
            h0T = const.tile([P, KO, B], F32, tag="h0T")
            nc.scalar.copy(h0T[:], ps_h[:, 0:KO, :])
            h0hh = const.tile([P, KO, B], F16, tag="h0hh")
            nc.scalar.mul(h0hh[:], h0T[:], 0.5)

            # ---- gh = h0 @ W_hh.T + b_hh (fp32, exact; step-invariant) ----
            # Cpre[:, 0:8]  = gh_rz + b_ih_rz            (r/z gate constants)
            # Cpre[:, 8:12] = 0.5*gh_n + b_ih_n  (n-group PSUM preload, where
            #                 0.5*gh_n comes from r = (1+tanh)/2 expansion)
            ps_g = psg.tile([P, 16, B], F32, tag="gates")
            nc.vector.tensor_copy(
                ps_g[:, 0:GM, :], bhh_sb[:, :, None].to_broadcast((P, GM, B))
            )
            for kc in range(2):
                wchunk2 = stream.tile([P, 2, 3 * HID], F32, tag="wst")
                nc.sync.dma_start(wchunk2[:], WHHP[:, 2 * kc:2 * kc + 2, :])
                for j in range(2):
                    k = 2 * kc + j
                    for m in range(GM):
                        nc.tensor.matmul(
                            ps_g[:, m, :],
                            wchunk2[:, j, m * P:(m + 1) * P],
                            h0T[:, k, :],
                            start=False, stop=(k == KO - 1),
                            skip_group_check=True,
                        )
            # ps_g now holds gh (bias b_hh included)
            Cpre = const.tile([P, GM, B], F32, tag="Cpre")
            for m in range(8):
                nc.scalar.activation(
                    Cpre[:, m, :], ps_g[:, m, :], AF.Identity,
                    bias=bih_sb[:, m, None], scale=1.0,
                )
            hn2f = const.tile([P, KO, B], F32, tag="hn2f")
            nc.scalar.mul(hn2f[:], ps_g[:, 8:GM, :], 0.5)
            for i in range(KO):
                nc.scalar.activation(
                    Cpre[:, 8 + i, :], hn2f[:, i, :], AF.Identity,
                    bias=bih_sb[:, 8 + i, None], scale=1.0,
                )
            hn2h = const.tile([P, KO, B], F16, tag="hn2h")
            nc.vector.tensor_copy(hn2h[:], hn2f[:])

            # ---- interleaved GRU + vocab projection ----
            MM_ORDER = [0, 1, 2, 3, 8, 9, 10, 11, 4, 5, 6, 7]  # r, n, z

            unit_q = collections.deque()
            ob_tiles = {}

            def emit_unit():
                g, m = unit_q.popleft()
                t0, t1 = GROUPS[g]
                ts = t1 - t0
                if m == 0:
                    ob_tiles[g] = obp.tile(
                        [P, 15, 16, B], F16, tag="ob", name=f"ob{g}a"
                    )
                    ob_tiles[g + 100] = obp.tile(
                        [P, 15, 16, B], F16, tag="ob", name=f"ob{g}b"
                    )
                ob = ob_tiles[g + (100 if m >= 15 else 0)]
                pp = psp.tile([P, 16, B], F32, tag="pp")
                ps = pp[:, :ts, :]
                for k in range(KO):
                    nc.tensor.matmul(
                        ps,
                        wout[:, k, m * P:(m + 1) * P],
                        resB[:, 1 + t0:1 + t1, k, :],
                        start=(k == 0), stop=(k == KO - 1),
                    )
                dst = ob[:, m % 15, :ts, :]
                if m % 2 == 0:
                    nc.scalar.activation(
                        dst, ps, AF.Identity, bias=bout_sb[:, m, None], scale=1.0
                    )
                else:
                    nc.vector.tensor_scalar_add(dst, ps, bout_sb[:, m, None])
                if m == 14 or m == MT - 1:
                    half = 0 if m == 14 else 1
                    nc.sync.dma_start(
                        OUT[:, 15 * half:15 * half + 15, t0:t1, :],
                        ob[:, :, :ts, :],
                    )

            gi = 0
            for t in range(STEPS):
                # ---- GRU step t: reads resB[:, t], writes resB[:, t+1] ----
                gp = psg.tile([P, 16, B], F32, tag="gates")
                nc.scalar.copy(gp[:, 0:GM, :], Cpre[:])  # PSUM preload
                for m in MM_ORDER:
                    for k in range(KO):
                        nc.tensor.matmul(
                            gp[:, m, :], wih[:, k, m, :], resB[:, t, k, :],
                            start=False, stop=(k == KO - 1),
                            skip_group_check=True,
                        )
                tr = sp.tile([P, KO, B], F16, tag="tr")
                nc.scalar.activation(tr[:], gp[:, 0:4, :], AF.Tanh, scale=0.5)
                tz = sp.tile([P, KO, B], F16, tag="tz")
                nc.scalar.activation(tz[:], gp[:, 4:8, :], AF.Tanh, scale=0.5)
                a = sp.tile([P, KO, B], F16, tag="a")
                nc.vector.tensor_mul(a[:], tr[:], hn2h[:])
                nc.vector.tensor_add(gp[:, 12:16, :], gp[:, 8:12, :], a[:])
                nT = sp.tile([P, KO, B], F16, tag="nT")
                nc.scalar.activation(nT[:], gp[:, 12:16, :], AF.Tanh, scale=1.0)
                # w0 = (tz + 1) * (0.5*h0)    [off critical path, Pool]
                w0t = sp.tile([P, KO, B], F16, tag="w0t")
                nc.gpsimd.scalar_tensor_tensor(
                    w0t[:], tz[:], 1.0, h0hh[:], ALU.add, ALU.mult
                )
                # y = (tz - 1) * n ; h' = -0.5*y + w0
                yt = sp.tile([P, KO, B], F16, tag="yt")
                nc.vector.scalar_tensor_tensor(
                    yt[:], tz[:], 1.0, nT[:], ALU.subtract, ALU.mult
                )
                nc.vector.scalar_tensor_tensor(
                    resB[:, t + 1, :, :], yt[:], -0.5, w0t[:], ALU.mult, ALU.add
                )

                # ---- interleave projection work ----
                if gi < len(GROUPS) and GROUPS[gi][1] == t:
                    unit_q.extend((gi, m) for m in range(MT))
                    gi += 1
                drain = 2 if len(unit_q) < 25 else 3
                for _ in range(drain):
                    if unit_q:
                        emit_unit()

            while gi < len(GROUPS):
                unit_q.extend((gi, m) for m in range(MT))
                gi += 1
            while unit_q:
                emit_unit()

    nc.compile()
    return nc


def _shard_inputs(feat, W_hp, b_hp, W_ih, W_hh, b_ih, b_hh, embed, W_out, b_out):
    f16 = ml_dtypes.float16 if hasattr(ml_dtypes, "float16") else np.float16
    f32 = np.float32

    def pk(x, parts):  # [(k p), rest] -> [p, k, rest]
        x = np.asarray(x)
        return np.ascontiguousarray(
            x.reshape(parts, P, *x.shape[1:]).transpose(1, 0, *range(2, x.ndim + 1))
        )

    featP = pk(np.asarray(feat, f32).T, KF)                     # [P, KF, B]
    whpP = pk(np.asarray(W_hp, f32).T, KF)                      # [P, KF, HID]
    whhP = pk(np.asarray(W_hh, f32).T, KO)                      # [P, KO, 3H]
    wihP = pk(np.asarray(W_ih, f32).T, KO).reshape(P, KO, GM, P).astype(f16)
    bihP = np.ascontiguousarray(np.asarray(b_ih, f32).reshape(GM, P).T)
    bhhP = np.ascontiguousarray(np.asarray(b_hh, f32).reshape(GM, P).T)
    bhpP = np.ascontiguousarray(np.asarray(b_hp, f32).reshape(KO, P).T)
    x0 = np.asarray(embed)[SOS].astype(f32).reshape(KO, P).T    # [P, KO]
    x0P = np.ascontiguousarray(
        np.repeat(x0[:, :, None], B, axis=2)
    ).astype(f16)                                               # [P, KO, B]

    Wo = np.zeros((NCORES * VPAD, HID), f32)
    Wo[:VOCAB] = W_out
    bo = np.zeros((NCORES * VPAD,), f32)
    bo[:VOCAB] = b_out
    common = dict(
        FEATP=featP, WHPP=whpP, WHHP=whhP, WIHP=wihP,
        BIHP=bihP, BHHP=bhhP, BHPP=bhpP, X0P=x0P,
    )
    in_maps = []
    for c in range(NCORES):
        sl = slice(c * VPAD, (c + 1) * VPAD)
        m = dict(common)
        m["WOUTP"] = pk(np.ascontiguousarray(Wo[sl].T), KO).astype(f16)
        m["BOUTP"] = np.ascontiguousarray(bo[sl].reshape(MT, P).T)
        in_maps.append(m)
    return in_maps


def kernel(**inputs):
    global LAST_RESULTS
    args = {k: np.asarray(v) for k, v in inputs.items()}
    in_maps = _shard_inputs(
        args["feat"], args["W_hp"], args["b_hp"], args["W_ih"], args["W_hh"],
        args["b_ih"], args["b_hh"], args["embed"], args["W_out"], args["b_out"],
    )
    nc = build()
    res = run_bass_kernel_spmd(nc, in_maps, core_ids=list(range(NCORES)))
    LAST_RESULTS = res
    # per-core OUT: [P, MT, T, B] fp16; vocab row = m*P + p
    shards = []
    for r in res.results:
        arr = np.asarray(r["OUT"])                      # [128, 30, 200, 32]
        shards.append(arr.transpose(3, 1, 0, 2).reshape(B, VPAD, STEPS))
    out = np.concatenate(shards, axis=1)[:, :VOCAB, :]
    return np.ascontiguousarray(out, dtype=np.float32)


# revision 7
# speedup vs baseline: 1.7667x; 1.0414x over previous
"""Trainium2 Bass kernel for the GRU caption model.

Computes: h0 = feat @ W_hp.T + b_hp; 200-step GRU whose hidden-proj gate
pre-activations are step-invariant; logits = outs @ W_out.T + b_out -> [B, V, T].

Strategy (v3):
- Every core runs the (tiny, latency-bound) GRU redundantly; the vocab dim of
  W_out/b_out is sharded 8 ways; each core emits a [3840, T, B] fp16 logits
  shard which the host transposes/concatenates.
- All recurrent state and 2-byte operands are fp16 (1 cyc/row matmuls, 2x/4x
  DVE modes, half DMA bytes); h0/gh are computed exactly in fp32, and the
  step-invariant gate constants are PRE-LOADED into PSUM so the matmuls
  accumulate on top (start=False), removing the "+C" adds from the chain.
- The gate PSUM is split into two tiles (r | z+n) so the r-gate tanh is only
  bank-serialized against the 16 r matmuls, not all 48 (PSUM deps are
  bank-level); sn2 goes to SBUF to dodge another bank serialization.
- Per step chain: r-mms -> tanh(r) [Act] -> mul, add [DVE] -> tanh(n) [Act]
  -> 2 fp16 tensor_tensor ops [DVE, 2x mode] -> h', written straight into the
  fp16 time-major state buffer read by both next-step matmuls and the vocab
  projection. z-gate products are computed off-chain on Pool.
- The vocab projection (bulk of PE work) is chopped into (<=16-step x 30
  vocab-tile) units and interleaved between GRU steps so PE stays dense; PSUM
  results are copied (+bias, fp16) to SBUF on Act/DVE alternately and written
  out with a few large fully-coalesced DMAs.
"""

import collections

import numpy as np
import ml_dtypes

import concourse.bass as bass
import concourse.mybir as mybir
import concourse.tile as tile
from concourse import bacc
from concourse.bass_utils import run_bass_kernel_spmd

F32 = mybir.dt.float32
F16 = mybir.dt.float16
AF = mybir.ActivationFunctionType
ALU = mybir.AluOpType

VOCAB = 30522
HID = 512
FEAT = 2048
STEPS = 200
B = 32
SOS = 101
NCORES = 8
P = 128
KO = HID // P           # 4 h-chunks
GM = 3 * HID // P       # 12 gate row-groups (r: 0-3, z: 4-7, n: 8-11)
KF = FEAT // P          # 16 feat chunks
VPAD = 3840             # per-core padded vocab rows = 30 * 128
MT = VPAD // P          # 30 vocab tiles per core

# projection column groups: small head groups (fill PE early), 16-step slabs,
# a 12-step tail group (shortens the post-loop drain)
GROUPS = [(0, 4), (4, 12)] + [(t, t + 16) for t in range(12, 188, 16)] + [(188, 200)]
assert GROUPS[-1][1] == STEPS and all(b - a <= 16 for a, b in GROUPS)

LAST_RESULTS = None  # test harness introspection


def build():
    nc = bacc.Bacc("TRN2", target_bir_lowering=False, debug=False)

    FEATP = nc.dram_tensor("FEATP", [P, KF, B], F32, kind="ExternalInput")
    WHPP = nc.dram_tensor("WHPP", [P, KF, HID], F32, kind="ExternalInput")
    WHHP = nc.dram_tensor("WHHP", [P, KO, 3 * HID], F32, kind="ExternalInput")
    WIHP = nc.dram_tensor("WIHP", [P, KO, GM, P], F16, kind="ExternalInput")
    WOUTP = nc.dram_tensor("WOUTP", [P, KO, VPAD], F16, kind="ExternalInput")
    BIHP = nc.dram_tensor("BIHP", [P, GM], F32, kind="ExternalInput")
    BHHP = nc.dram_tensor("BHHP", [P, GM], F32, kind="ExternalInput")
    BHPP = nc.dram_tensor("BHPP", [P, KO], F32, kind="ExternalInput")
    BOUTP = nc.dram_tensor("BOUTP", [P, MT], F32, kind="ExternalInput")
    X0P = nc.dram_tensor("X0P", [P, KO, B], F16, kind="ExternalInput")
    OUT = nc.dram_tensor("OUT", [P, MT, STEPS, B], F16, kind="ExternalOutput")

    with tile.TileContext(nc) as tc:
        with (
            tc.tile_pool(name="const", bufs=1) as const,
            tc.tile_pool(name="stream", bufs=2) as stream,
            tc.tile_pool(name="step", bufs=3) as sp,
            tc.tile_pool(name="obp", bufs=2) as obp,
            tc.tile_pool(name="psg", bufs=2, space="PSUM") as psg,
            tc.tile_pool(name="psp", bufs=4, space="PSUM") as psp,
        ):
            # ---- loads needed by the h0/gh phase first ----
            featT = const.tile([P, KF, B], F32, tag="featT")
            nc.sync.dma_start(featT[:], FEATP[:, :, :])
            bih_sb = const.tile([P, GM], F32, tag="bih")
            nc.sync.dma_start(bih_sb[:], BIHP[:, :])
            bhh_sb = const.tile([P, GM], F32, tag="bhh")
            nc.sync.dma_start(bhh_sb[:], BHHP[:, :])
            bhp_sb = const.tile([P, KO], F32, tag="bhp")
            nc.sync.dma_start(bhp_sb[:], BHPP[:, :])
            bout_sb = const.tile([P, MT], F32, tag="bout")
            nc.sync.dma_start(bout_sb[:], BOUTP[:, :])

            # recurrent state, time-major: resB[:, t+1] = h_t ; resB[:, 0] = x0
            resB = const.tile([P, STEPS + 1, KO, B], F16, tag="resB")
            nc.sync.dma_start(resB[:, 0, :, :], X0P[:, :, :])

            # ---- h0 = feat @ W_hp.T + b_hp (fp32, exact) ----
            # b_hp is PRE-LOADED into PSUM; all matmuls accumulate (start=False)
            ps_h = psg.tile([P, KO, B], F32, tag="gr")
            nc.vector.tensor_copy(
                ps_h[:], bhp_sb[:, :, None].to_broadcast((P, KO, B))
            )
            for kc in range(4):
                wchunk = stream.tile([P, 4, HID], F32, tag="wst")
                nc.sync.dma_start(wchunk[:], WHPP[:, 4 * kc:4 * kc + 4, :])
                for j in range(4):
                    kf = 4 * kc + j
                    for ko in range(KO):
                        nc.tensor.matmul(
                            ps_h[:, ko, :],
                            wchunk[:, j, ko * P:(ko + 1) * P],
                            featT[:, kf, :],
                            start=False, stop=(kf == KF - 1),
                            skip_group_check=True,
                        )
            h0T = const.tile([P, KO, B], F32, tag="h0T")
            nc.scalar.copy(h0T[:], ps_h[:])
            h0hh = const.tile([P, KO, B], F16, tag="h0hh")
            nc.scalar.mul(h0hh[:], h0T[:], 0.5)

            # ---- gh = h0 @ W_hh.T + b_hh (fp32, exact; step-invariant) ----
            ps_ga = psg.tile([P, 8, B], F32, tag="gzn")    # gh groups 0..7 (r,z)
            nc.vector.tensor_copy(
                ps_ga[:], bhh_sb[:, 0:8, None].to_broadcast((P, 8, B))
            )
            ps_gb = psg.tile([P, KO, B], F32, tag="gr")    # gh groups 8..11 (n)
            nc.vector.tensor_copy(
                ps_gb[:], bhh_sb[:, 8:GM, None].to_broadcast((P, KO, B))
            )
            for kc in range(2):
                wchunk2 = stream.tile([P, 2, 3 * HID], F32, tag="wst")
                nc.sync.dma_start(wchunk2[:], WHHP[:, 2 * kc:2 * kc + 2, :])
                for j in range(2):
                    k = 2 * kc + j
                    for m in range(GM):
                        dst = ps_ga[:, m, :] if m < 8 else ps_gb[:, m - 8, :]
                        nc.tensor.matmul(
                            dst,
                            wchunk2[:, j, m * P:(m + 1) * P],
                            h0T[:, k, :],
                            start=False, stop=(k == KO - 1),
                            skip_group_check=True,
                        )
            # remaining resident weights (not needed until the loop / t>=4)
            wih = const.tile([P, KO, GM, P], F16, tag="wih")
            nc.sync.dma_start(wih[:], WIHP[:, :, :, :])
            wout = const.tile([P, KO, VPAD], F16, tag="wout")
            nc.sync.dma_start(wout[:], WOUTP[:, :, :])

            # Cpre_r = gh_r + b_ih_r                       (r PSUM preload)
            # Cpre_zn[0:4] = gh_z + b_ih_z                 (z PSUM preload)
            # Cpre_zn[4:8] = 0.5*gh_n + b_ih_n             (n PSUM preload; the
            #                0.5 comes from r = (1+tanh)/2 expansion)
            Cpre_r = const.tile([P, KO, B], F32, tag="Cpre_r")
            for m in range(4):
                nc.scalar.activation(
                    Cpre_r[:, m, :], ps_ga[:, m, :], AF.Identity,
                    bias=bih_sb[:, m, None], scale=1.0,
                )
            Cpre_zn = const.tile([P, 8, B], F32, tag="Cpre_zn")
            for m in range(4):
                nc.scalar.activation(
                    Cpre_zn[:, m, :], ps_ga[:, 4 + m, :], AF.Identity,
                    bias=bih_sb[:, 4 + m, None], scale=1.0,
                )
            hn2f = const.tile([P, KO, B], F32, tag="hn2f")
            nc.scalar.mul(hn2f[:], ps_gb[:], 0.5)
            for i in range(KO):
                nc.scalar.activation(
                    Cpre_zn[:, 4 + i, :], hn2f[:, i, :], AF.Identity,
                    bias=bih_sb[:, 8 + i, None], scale=1.0,
                )
            hn2h = const.tile([P, KO, B], F16, tag="hn2h")
            nc.vector.tensor_copy(hn2h[:], hn2f[:])

            # ---- interleaved GRU + vocab projection ----
            unit_q = collections.deque()
            ob_tiles = {}

            def emit_unit():
                g, m = unit_q.popleft()
                t0, t1 = GROUPS[g]
                ts = t1 - t0
                if m == 0:
                    ob_tiles[g] = obp.tile(
                        [P, 15, 16, B], F16, tag="ob", name=f"ob{g}a"
                    )
                    ob_tiles[g + 100] = obp.tile(
                        [P, 15, 16, B], F16, tag="ob", name=f"ob{g}b"
                    )
                ob = ob_tiles[g + (100 if m >= 15 else 0)]
                pp = psp.tile([P, 16, B], F32, tag="pp")
                ps = pp[:, :ts, :]
                for k in range(KO):
                    nc.tensor.matmul(
                        ps,
                        wout[:, k, m * P:(m + 1) * P],
                        resB[:, 1 + t0:1 + t1, k, :],
                        start=(k == 0), stop=(k == KO - 1),
                    )
                dst = ob[:, m % 15, :ts, :]
                if m % 2 == 0:
                    nc.scalar.activation(
                        dst, ps, AF.Identity, bias=bout_sb[:, m, None], scale=1.0
                    )
                else:
                    nc.vector.tensor_scalar_add(dst, ps, bout_sb[:, m, None])
                if m == 14 or m == MT - 1:
                    half = 0 if m == 14 else 1
                    nc.sync.dma_start(
                        OUT[:, 15 * half:15 * half + 15, t0:t1, :],
                        ob[:, :, :ts, :],
                    )

            gi = 0
            for t in range(STEPS):
                # ---- GRU step t: reads resB[:, t], writes resB[:, t+1] ----
                gr = psg.tile([P, KO, B], F32, tag="gr")     # r gates
                gzn = psg.tile([P, 8, B], F32, tag="gzn")    # z | n gates
                nc.scalar.copy(gr[:], Cpre_r[:])             # PSUM preloads
                nc.vector.tensor_copy(gzn[:], Cpre_zn[:])
                for i, (dst_m, wm) in enumerate(
                    [(gr[:, m, :], m) for m in range(4)]
                    + [(gzn[:, m - 4, :], m) for m in range(4, 12)]
                ):
                    for k in range(KO):
                        nc.tensor.matmul(
                            dst_m, wih[:, k, wm, :], resB[:, t, k, :],
                            start=False, stop=(k == KO - 1),
                            skip_group_check=True,
                        )
                tr = sp.tile([P, KO, B], F16, tag="tr")
                nc.scalar.activation(tr[:], gr[:], AF.Tanh, scale=0.5)
                tz = sp.tile([P, KO, B], F16, tag="tz")
                nc.scalar.activation(tz[:], gzn[:, 0:4, :], AF.Tanh, scale=0.5)
                # off-chain z products on Pool:
                #   hm = 0.5 - 0.5*tz ; w0 = (0.5 + 0.5*tz) * h0  (as um*h0b)
                hm = sp.tile([P, KO, B], F16, tag="hm")
                nc.gpsimd.tensor_scalar(hm[:], tz[:], -0.5, 0.5, ALU.mult, ALU.add)
                um = sp.tile([P, KO, B], F16, tag="um")
                nc.gpsimd.tensor_scalar(um[:], tz[:], 0.5, 0.5, ALU.mult, ALU.add)
                w0t = sp.tile([P, KO, B], F16, tag="w0t")
                nc.gpsimd.tensor_mul(w0t[:], um[:], h0hh[:])
                # n-gate chain
                a = sp.tile([P, KO, B], F16, tag="a")
                nc.vector.tensor_mul(a[:], tr[:], hn2h[:])
                snb = sp.tile([P, KO, B], F32, tag="snb")
                nc.vector.tensor_add(snb[:], gzn[:, 4:8, :], a[:])
                nT = sp.tile([P, KO, B], F16, tag="nT")
                nc.scalar.activation(nT[:], snb[:], AF.Tanh, scale=1.0)
                # h' = hm*n + w0
                tm = sp.tile([P, KO, B], F16, tag="tm")
                nc.vector.tensor_mul(tm[:], hm[:], nT[:])
                nc.vector.tensor_add(resB[:, t + 1, :, :], tm[:], w0t[:])

                # ---- interleave projection work ----
                if gi < len(GROUPS) and GROUPS[gi][1] == t:
                    unit_q.extend((gi, m) for m in range(MT))
                    gi += 1
                drain = 3 if len(unit_q) > 34 else 2
                for _ in range(drain):
                    if unit_q:
                        emit_unit()

            while gi < len(GROUPS):
                unit_q.extend((gi, m) for m in range(MT))
                gi += 1
            while unit_q:
                emit_unit()

    nc.compile()
    return nc


def _shard_inputs(feat, W_hp, b_hp, W_ih, W_hh, b_ih, b_hh, embed, W_out, b_out):
    f16 = np.float16
    f32 = np.float32

    def pk(x, parts):  # [(k p), rest] -> [p, k, rest]
        x = np.asarray(x)
        return np.ascontiguousarray(
            x.reshape(parts, P, *x.shape[1:]).transpose(1, 0, *range(2, x.ndim + 1))
        )

    featP = pk(np.asarray(feat, f32).T, KF)                     # [P, KF, B]
    whpP = pk(np.asarray(W_hp, f32).T, KF)                      # [P, KF, HID]
    whhP = pk(np.asarray(W_hh, f32).T, KO)                      # [P, KO, 3H]
    wihP = pk(np.asarray(W_ih, f32).T, KO).reshape(P, KO, GM, P).astype(f16)
    bihP = np.ascontiguousarray(np.asarray(b_ih, f32).reshape(GM, P).T)
    bhhP = np.ascontiguousarray(np.asarray(b_hh, f32).reshape(GM, P).T)
    bhpP = np.ascontiguousarray(np.asarray(b_hp, f32).reshape(KO, P).T)
    x0 = np.asarray(embed)[SOS].astype(f32).reshape(KO, P).T    # [P, KO]
    x0P = np.ascontiguousarray(
        np.repeat(x0[:, :, None], B, axis=2)
    ).astype(f16)                                               # [P, KO, B]

    Wo = np.zeros((NCORES * VPAD, HID), f32)
    Wo[:VOCAB] = W_out
    bo = np.zeros((NCORES * VPAD,), f32)
    bo[:VOCAB] = b_out
    common = dict(
        FEATP=featP, WHPP=whpP, WHHP=whhP, WIHP=wihP,
        BIHP=bihP, BHHP=bhhP, BHPP=bhpP, X0P=x0P,
    )
    in_maps = []
    for c in range(NCORES):
        sl = slice(c * VPAD, (c + 1) * VPAD)
        m = dict(common)
        m["WOUTP"] = pk(np.ascontiguousarray(Wo[sl].T), KO).astype(f16)
        m["BOUTP"] = np.ascontiguousarray(bo[sl].reshape(MT, P).T)
        in_maps.append(m)
    return in_maps


def kernel(**inputs):
    global LAST_RESULTS
    args = {k: np.asarray(v) for k, v in inputs.items()}
    in_maps = _shard_inputs(
        args["feat"], args["W_hp"], args["b_hp"], args["W_ih"], args["W_hh"],
        args["b_ih"], args["b_hh"], args["embed"], args["W_out"], args["b_out"],
    )
    nc = build()
    res = run_bass_kernel_spmd(nc, in_maps, core_ids=list(range(NCORES)))
    LAST_RESULTS = res
    # per-core OUT: [P, MT, T, B] fp16; vocab row = m*P + p
    shards = []
    for r in res.results:
        arr = np.asarray(r["OUT"])                      # [128, 30, 200, 32]
        shards.append(arr.transpose(3, 1, 0, 2).reshape(B, VPAD, STEPS))
    out = np.concatenate(shards, axis=1)[:, :VOCAB, :]
    return np.ascontiguousarray(out, dtype=np.float32)
